# revision 31
# baseline (speedup 1.0000x reference)
"""Multi-head attention (B=2, S=2048, D=1024, H=16, d=64) on 8 TRN2 NeuronCores.

Sharding: core i handles batch b = i // 4 and query rows [qb*512, (qb+1)*512)
with qb = i % 4. No collectives: each core computes K/V for its whole batch,
attention for its query block, and the full output projection for its rows.

v2 design (vs baseline):
  - groups of 2 t-chunks: 2 scores MMs -> 1 exp (FD=1024) -> 2 AV MMs
    (pipelined one deep, QKV filler keeps the PE busy during exp).
  - out-projection runs as filler thunks (DVE-accumulated into ySb), delayed
    one pair behind the Zb/reciprocal round-trip so the PE queue never stalls
    on it; the last two pairs' thunks form the (short) tail.
  - 1/Z via reciprocal_approx_fast (~5x faster than reciprocal).
  - V stored per pair as [128, 16, 132] blocks [V0|1|pad|V1|1|pad] so the
    PSUM->SBUF evacuation is one aligned strided cast per 8 t-chunks and the
    AV lhsT slices stay contiguous 65-column reads (ones-column computes Z).
  - K/Q projections evacuate as single chunky bf16 casts ([128,2048] resp.
    both heads col-stacked in one [128,512] tile).
"""

import math
from collections import deque
from contextlib import ExitStack
from functools import lru_cache

import ml_dtypes
import numpy as np

import concourse.bass as bass
import concourse.bacc as bacc
import concourse.mybir as mybir
import concourse.tile as tile

BF16 = mybir.dt.bfloat16
F32 = mybir.dt.float32
NPBF16 = ml_dtypes.bfloat16

B, S, D, H, d = 2, 2048, 1024, 16, 64
NCORES = 8
QBLKS = 4              # query blocks per batch
QB = S // QBLKS        # 512 query rows per core
NP = H // 2            # 8 head pairs
TCH = S // 128         # 16 t-chunks of 128
NG = 8                 # groups per (pair, dlt); 2 t-chunks each
SCALE = 1.0 / math.sqrt(d)
EXP = mybir.ActivationFunctionType.Exp


def build_nc() -> bass.Bass:
    nc = bacc.Bacc("TRN2", target_bir_lowering=False, debug=False)

    xT_d = nc.dram_tensor("xT", [D, S], BF16, kind="ExternalInput").ap()
    xTqa_d = nc.dram_tensor("xTqa", [H, 65, QB], BF16, kind="ExternalInput").ap()
    wqa_d = nc.dram_tensor("wqa", [H, 65, 64], BF16, kind="ExternalInput").ap()
    wk_d = nc.dram_tensor("wk_blk", [NP, 128, 128], BF16, kind="ExternalInput").ap()
    wv_d = nc.dram_tensor("wv_blk", [NP, 128, 128], BF16, kind="ExternalInput").ap()
    woT_d = nc.dram_tensor("woT", [D, D], BF16, kind="ExternalInput").ap()
    bo2_d = nc.dram_tensor("bo2", [1, D], F32, kind="ExternalInput").ap()
    y_d = nc.dram_tensor("y", [QB, D], F32, kind="ExternalOutput").ap()

    rdr_d = nc.dram_tensor("rscratch", [NP, 2, QB], F32).ap()

    with ExitStack() as ctx:
        tc = ctx.enter_context(tile.TileContext(nc))
        persist = ctx.enter_context(tc.tile_pool(name="persist", bufs=1))

        wqa_all = persist.tile([65, H * 64], BF16, name="wqa", tag="wqa")
        wk_all = persist.tile([128, NP * 128], BF16, name="wk", tag="wk")
        wv_all = persist.tile([128, NP * 128], BF16, name="wv", tag="wv")
        xTqa_all = persist.tile([65, H * QB], BF16, name="xTqa", tag="xTqa")
        wqa_sb = [wqa_all[:, h * 64 : (h + 1) * 64] for h in range(H)]
        wk_sb = [wk_all[:, p * 128 : (p + 1) * 128] for p in range(NP)]
        wv_sb = [wv_all[:, p * 128 : (p + 1) * 128] for p in range(NP)]
        xTqa_sb = [xTqa_all[:, h * QB : (h + 1) * QB] for h in range(H)]
        boB_sb = persist.tile([128, D], F32, name="boB", tag="boB")
        woT_sb = [persist.tile([128, D], BF16, name=f"woT{p}", tag=f"woT{p}") for p in range(NP)]
        qT_sb = [persist.tile([128, QB], BF16, name=f"qT{p}", tag=f"qT{p}") for p in range(NP)]
        kT_sb = [persist.tile([128, S], BF16, name=f"kT{p}", tag=f"kT{p}") for p in range(NP)]
        # V per pair: 16 blocks of 132 cols: [V0(64) | 1 | pad | V1(64) | 1 | pad]
        vv_sb = [persist.tile([128, TCH * 132], BF16, name=f"vv{p}", tag=f"vv{p}") for p in range(NP)]
        outT_sb = [persist.tile([128, QB], BF16, name=f"outT{p}", tag=f"outT{p}") for p in range(NP)]
        warm_sb = persist.tile([128, 512], BF16, name="warm", tag="warm")

        with (
            tc.tile_pool(name="xTpool", bufs=1) as xpool,
            tc.tile_pool(name="pps", bufs=2, space="PSUM") as pps,     # scores f32 [128,1024] = 2 banks
            tc.tile_pool(name="ppav", bufs=2, space="PSUM") as ppav,   # av f32 [65,512] = 1 bank
            tc.tile_pool(name="ppf", bufs=1, space="PSUM") as ppf,     # filler f32 [128,1024] = 2 banks
            tc.tile_pool(name="eTp", bufs=6) as eTp,
            tc.tile_pool(name="rbp", bufs=3) as rbp,
        ):
            xT_sb = [xpool.tile([128, S], BF16, name=f"xT{p}", tag=f"xT{p}") for p in range(NP)]
            # HAM warm-up: dependency-free matmul spin on scratch data keeps
            # the PE busy through the input-DMA ramp so it unthrottles early
            nc.vector.memset(warm_sb[:], 1.0)
            for i in range(24):
                pw = pps.tile([128, 1024], F32, name="ps", tag="ps")
                nc.tensor.matmul(pw[:, 0:512], warm_sb[:, 0:128], warm_sb[:], start=True, stop=True)
                nc.tensor.matmul(pw[:, 512:1024], warm_sb[:, 0:128], warm_sb[:], start=True, stop=True)
            for p in range(NP):
                for dlt in range(2):
                    h = 2 * p + dlt
                    nc.sync.dma_start(out=wqa_sb[h], in_=wqa_d[h])
                    nc.sync.dma_start(out=xTqa_sb[h], in_=xTqa_d[h])
                nc.sync.dma_start(out=wk_sb[p], in_=wk_d[p])
                nc.sync.dma_start(out=wv_sb[p], in_=wv_d[p])
                nc.sync.dma_start(out=xT_sb[p][:], in_=xT_d[p * 128 : (p + 1) * 128, :])
            for p in range(NP):
                nc.sync.dma_start(out=woT_sb[p][:], in_=woT_d[p * 128 : (p + 1) * 128, :])
            nc.sync.dma_start(
                out=boB_sb[:],
                in_=bass.AP(tensor=bo2_d.tensor, offset=bo2_d.offset, ap=[[0, 128], [1, D]]),
            )

            def qkv_thunks(p):
                th = []

                def _k(p, kh):
                    pk = ppf.tile([128, 1024], F32, name="pf", tag="pf")
                    for ck in range(2):
                        nc.tensor.matmul(
                            pk[:, ck * 512 : (ck + 1) * 512],
                            wk_sb[p][:],
                            xT_sb[p][:, (kh * 2 + ck) * 512 : (kh * 2 + ck + 1) * 512],
                            start=True,
                            stop=True,
                        )
                    nc.vector.tensor_copy(
                        kT_sb[p][:, kh * 1024 : (kh + 1) * 1024], pk[:]
                    )

                def _q(p):
                    pq = ppf.tile([128, 1024], F32, name="pf", tag="pf")
                    for dlt in range(2):
                        h = 2 * p + dlt
                        nc.tensor.matmul(
                            pq[dlt * 64 : (dlt + 1) * 64, 0:512],
                            wqa_sb[h][:],
                            xTqa_sb[h][:],
                            start=True,
                            stop=True,
                        )
                    nc.vector.tensor_copy(qT_sb[p][:], pq[:, 0:512])

                def _v(p, g8):
                    pv = ppf.tile([128, 1024], F32, name="pf", tag="pf")
                    for c in range(8):
                        t = g8 * 8 + c
                        nc.tensor.matmul(
                            pv[:, c * 128 : (c + 1) * 128],
                            xT_sb[p][:, t * 128 : (t + 1) * 128],
                            wv_sb[p][:],
                            start=True,
                            stop=True,
                        )
                    vt = vv_sb[p][:]
                    nc.vector.tensor_copy(
                        bass.AP(
                            tensor=vt.tensor,
                            offset=vt.offset + g8 * 8 * 132,
                            ap=[vt.ap[0], [132, 8], [66, 2], [1, 64]],
                        ),
                        bass.AP(
                            tensor=pv[:].tensor,
                            offset=pv[:].offset,
                            ap=[pv[:].ap[0], [128, 8], [64, 2], [1, 64]],
                        ),
                    )

                def _ones(p):
                    vt = vv_sb[p][:]
                    nc.vector.memset(
                        bass.AP(
                            tensor=vt.tensor,
                            offset=vt.offset + 64,
                            ap=[vt.ap[0], [132, TCH], [66, 2]],
                        ),
                        1.0,
                    )

                th.append(lambda p=p: _k(p, 0))
                th.append(lambda p=p: _v(p, 0))
                th.append(lambda p=p: _q(p))
                th.append(lambda p=p: _k(p, 1))
                th.append(lambda p=p: _v(p, 1))
                th.append(lambda p=p: _ones(p))
                return th

            for th in qkv_thunks(0):
                th()
            for i in range(6):
                pw = pps.tile([128, 1024], F32, name="ps", tag="ps")
                nc.tensor.matmul(pw[:, 0:512], warm_sb[:, 0:128], warm_sb[:], start=True, stop=True)
                nc.tensor.matmul(pw[:, 512:1024], warm_sb[:, 0:128], warm_sb[:], start=True, stop=True)
            for th in qkv_thunks(1):
                th()

            filler = deque()
            av_of = {}
            avP_of = {}
            pending = None

            def emit_pair_evac(p):
                avP = avP_of[p]
                # Z rows to DRAM, then broadcast both back as [128, 512]
                nc.sync.dma_start(out=rdr_d[p, 0], in_=avP[64:65, :])
                nc.vector.tensor_copy(avP[64:128, :], av_of[(p, 1)][0:64, :])
                zs = rbp.tile([1, QB], F32, name="zs", tag="zs")
                nc.vector.tensor_copy(zs[:], av_of[(p, 1)][64:65, :])
                nc.sync.dma_start(out=rdr_d[p, 1], in_=zs[:])
                Zb = rbp.tile([128, QB], F32, name="Zb", tag="Zb")
                src = rdr_d[p, 0]
                nc.sync.dma_start(
                    out=Zb[:],
                    in_=bass.AP(
                        tensor=src.tensor,
                        offset=src.offset,
                        ap=[[QB, 2], [0, 64], [1, QB]],
                    ),
                )
                Rb = rbp.tile([128, QB], F32, name="Rb", tag="Rb")
                nc.vector.reciprocal_approx_fast(out=Rb[:], in_=Zb[:])
                nc.vector.tensor_mul(outT_sb[p][:], avP[:], Rb[:])

            def emit_av(p, dlt, g, eT):
                av = av_of[(p, dlt)]
                for j in range(2):
                    t = g * 2 + j
                    vt = vv_sb[p][:]
                    lhsT = bass.AP(
                        tensor=vt.tensor,
                        offset=vt.offset + t * 132 + dlt * 66,
                        ap=[vt.ap[0], [1, 65]],
                    )
                    nc.tensor.matmul(
                        av[0:65, :],
                        lhsT,
                        eT[:, j * 512 : (j + 1) * 512],
                        start=(g == 0 and j == 0),
                        stop=(g == NG - 1 and j == 1),
                    )
                if g == NG - 1:
                    if dlt == 0:
                        avP = rbp.tile([128, QB], F32, name="avP", tag="avP")
                        avP_of[p] = avP
                        nc.vector.tensor_copy(avP[0:65, :], av[0:65, :])
                    else:
                        emit_pair_evac(p)

            for p in range(NP):
                if p + 2 < NP:
                    filler.extend(qkv_thunks(p + 2))
                for dlt in range(2):
                    av_of[(p, dlt)] = ppav.tile([65, QB], F32, name="av", tag="av")
                    klo = dlt * 64
                    for g in range(NG):
                        ps = pps.tile([128, 1024], F32, name="ps", tag="ps")
                        for j in range(2):
                            t = g * 2 + j
                            nc.tensor.matmul(
                                ps[:, j * 512 : (j + 1) * 512],
                                kT_sb[p][klo : klo + 64, t * 128 : (t + 1) * 128],
                                qT_sb[p][klo : klo + 64, :],
                                start=True,
                                stop=True,
                            )
                        eT = eTp.tile([128, 1024], BF16, name="eT", tag="eT")
                        nc.scalar.activation(eT[:], ps[:], EXP, scale=SCALE)
                        if pending is not None:
                            emit_av(*pending)
                        if filler:
                            filler.popleft()()
                        pending = (p, dlt, g, eT)
            emit_av(*pending)
            while filler:
                filler.popleft()()

        # ---------------- Epilogue: out-projection, PSUM-accumulated ----------------
        with (
            tc.tile_pool(name="epi", bufs=4, space="PSUM") as epool,
            tc.tile_pool(name="yop", bufs=2) as yop,
        ):
            for sc in range(QB // 128):
                for nk in range(D // 512):
                    py = epool.tile([128, 512], F32, name="py", tag="py")
                    for p in range(NP):
                        nc.tensor.matmul(
                            py[:],
                            outT_sb[p][:, sc * 128 : (sc + 1) * 128],
                            woT_sb[p][:, nk * 512 : (nk + 1) * 512],
                            start=(p == 0),
                            stop=(p == NP - 1),
                        )
                    yt = yop.tile([128, 512], F32, name="yt", tag="yt")
                    nc.vector.tensor_add(yt[:], py[:], boB_sb[:, nk * 512 : (nk + 1) * 512])
                    nc.sync.dma_start(
                        out=y_d[sc * 128 : (sc + 1) * 128, nk * 512 : (nk + 1) * 512],
                        in_=yt[:],
                    )

    nc.finalize()
    return nc


@lru_cache(maxsize=1)
def _cached_nc() -> bass.Bass:
    return build_nc()


def prepare_in_maps(embedding, Wq, Wk, Wv, bq, bk, bv, Wo, bo):
    """Host-side sharding/packing. Returns per-core input maps."""
    emb = np.asarray(embedding, dtype=np.float32)
    Wq = np.asarray(Wq, dtype=np.float32)
    Wk = np.asarray(Wk, dtype=np.float32)
    Wv = np.asarray(Wv, dtype=np.float32)
    bq = np.asarray(bq, dtype=np.float32)
    bk = np.asarray(bk, dtype=np.float32)
    bv = np.asarray(bv, dtype=np.float32)
    Wo = np.asarray(Wo, dtype=np.float32)
    bo = np.asarray(bo, dtype=np.float32)

    wk_blk = np.zeros([NP, 128, 128], np.float32)
    wv_blk = np.zeros([NP, 128, 128], np.float32)
    for p in range(NP):
        h0, h1 = 2 * p, 2 * p + 1
        wk_blk[p, 0:64, 0:64] = Wk[h0].T
        wk_blk[p, 64:128, 64:128] = Wk[h1].T
        wv_blk[p, 0:64, 0:64] = Wv[h0].T
        wv_blk[p, 64:128, 64:128] = Wv[h1].T
    # per-head augmented Q weights: rows 0:64 = Wq_h^T, row 64 = bq_h
    wqa = np.zeros([H, 65, 64], np.float32)
    for h in range(H):
        wqa[h, 0:64, :] = Wq[h].T
        wqa[h, 64, :] = bq[h]

    wqa16 = wqa.astype(NPBF16)
    wk16 = wk_blk.astype(NPBF16)
    wv16 = wv_blk.astype(NPBF16)
    woT16 = np.ascontiguousarray(Wo.T).astype(NPBF16)
    bo2 = (bo + Wo @ bv.reshape(-1)).reshape(1, D).astype(np.float32)

    xT_by_b = [np.ascontiguousarray(emb[b].T).astype(NPBF16) for b in range(B)]

    in_maps = []
    for core in range(NCORES):
        b, qb = core // QBLKS, core % QBLKS
        xT = xT_by_b[b]
        # per-head augmented xTq: rows 0:64 = embT rows of head h, row 64 = ones
        xTqa = np.ones([H, 65, QB], np.float32)
        for h in range(H):
            xTqa[h, 0:64, :] = xT[h * 64 : (h + 1) * 64, qb * QB : (qb + 1) * QB]
        in_maps.append(
            dict(
                xT=xT,
                xTqa=xTqa.astype(NPBF16),
                wqa=wqa16,
                wk_blk=wk16,
                wv_blk=wv16,
                woT=woT16,
                bo2=bo2,
            )
        )
    return in_maps


def assemble(results) -> np.ndarray:
    out = np.empty([B, S, D], np.float32)
    for core in range(NCORES):
        b, qb = core // QBLKS, core % QBLKS
        out[b, qb * QB : (qb + 1) * QB, :] = results[core]["y"]
    return out


def kernel(**inputs) -> np.ndarray:
    from concourse.bass_utils import run_bass_kernel_spmd

    in_maps = prepare_in_maps(**inputs)
    nc = _cached_nc()
    res = run_bass_kernel_spmd(nc, in_maps, list(range(NCORES)))
    return assemble(res.results)
